# revision 20
# baseline (speedup 1.0000x reference)
"""3-layer GAT + per-graph mean-pool + linear head, distributed over 8 NeuronCores.

Strategy (edge-parallel, dst-balanced, bf16):
  * Host: bin-pack nodes into 160 (core, window) bins of 128 nodes each so
    every window owns exactly ~2048 incoming edges (nblk=16 blocks of 128);
    remap node ids to (core, window, slot) order and sort edges by dst.
  * Per layer each core computes z|el|er for its own 2560 nodes (one matmul
    per window; attention logits folded in via Wel = W @ albd) into a bf16
    node table with 384-element rows [z(256)|el(4)|er(4)|pad]; the table is
    AllGather'd (Shared scratch) for layers 1-2; layer 0's table is computed
    fully replicated from the (replicated) input features.
  * Edge phase per window: two 1024-row dma_gather calls pull z_ext[src]
    rows (768 B each); er[dst] stays on-core (bf16 [128,4] per window) and is
    expanded to edges with one-hot matmuls whose selector selt is built with
    a gpsimd partition_broadcast of the host-provided dst row + one is_eq.
    ex = exp(leaky_relu(el+er)) is batched per window; messages are scaled in
    place and scatter-added with one-hot bf16 matmuls into PSUM [out|sum_ex].
    Softmax max-shift is dropped (shift-invariant; logits cannot overflow).
  * Pooling: per-window one-hot matmul accumulates per-graph sums over all
    64 graphs; a [64,256] AllReduce combines cores; every core computes the
    full logits and the host takes core 0's.
"""

import sys

import numpy as np

sys.path.insert(0, "/opt/trn_rl_repo")

import ml_dtypes

import concourse.bass as bass
import concourse.bacc as bacc
import concourse.mybir as mybir
import concourse.tile as tile
from concourse.bass_utils import run_bass_kernel_spmd

# Problem shape (hardcoded per contest rules).
N, E, G = 20480, 327680, 64
IN_DIM, H, D, C = 128, 4, 64, 10
HD = H * D            # 256
ROWT = 384            # bf16 table row: z(256) | el(4) | er(4) | pad(120)
NCORES = 8
RN = N // NCORES      # 2560 dst nodes per core
P = 128
NW = RN // P          # 20 windows per core
NPG = N // G          # 320 nodes per graph
NEG_SLOPE = 0.2
F32 = mybir.dt.float32
BF16 = mybir.dt.bfloat16
I16 = mybir.dt.int16
I32 = mybir.dt.int32
BF = ml_dtypes.bfloat16

TRACE = False         # set by test.py to capture HW profile
LAST_EXEC_NS = None
LAST_RESULTS = None

_CACHE = {}


def _install_ntff_hook_shim():
    """This image's ``antenv`` lacks ``axon_hooks``; provide the thin ctypes
    shim around libaxon_pjrt.so so run_bass_kernel_spmd(trace=True) works."""
    try:
        import antenv.axon_hooks  # noqa: F401
        return
    except ImportError:
        pass
    import contextlib
    import ctypes
    import types

    so_path = "/opt/axon/libaxon_pjrt.so"
    try:
        lib = ctypes.CDLL(so_path)
    except OSError:
        return
    if not hasattr(lib, "axon_start_nrt_profile"):
        return
    lib.axon_start_nrt_profile.argtypes = [ctypes.POINTER(ctypes.c_int64), ctypes.c_size_t]
    lib.axon_start_nrt_profile.restype = ctypes.c_int64
    lib.axon_stop_nrt_profile.argtypes = [ctypes.c_char_p]
    lib.axon_stop_nrt_profile.restype = ctypes.c_int64

    @contextlib.contextmanager
    def _hook(output_dir, device_ids):
        import jax

        jax.devices()
        if device_ids:
            ids = (ctypes.c_int64 * len(device_ids))(*device_ids)
            rc = lib.axon_start_nrt_profile(ids, len(device_ids))
        else:
            rc = lib.axon_start_nrt_profile(None, 0)
        if rc != 0:
            raise RuntimeError(f"axon_start_nrt_profile rc={rc}")
        try:
            yield
        finally:
            n = lib.axon_stop_nrt_profile(str(output_dir).encode())
            print(f"ntff profile: {n} file(s) written to {output_dir}")

    mod = types.ModuleType("antenv.axon_hooks")
    mod.get_axon_ntff_profile_hook = lambda: _hook
    mod.set_axon_ntff_profile_hook = lambda h: None
    sys.modules["antenv.axon_hooks"] = mod


# ----------------------------------------------------------------------------
# Host-side preprocessing (sharding / layout only)
# ----------------------------------------------------------------------------
def _pack_bins(deg):
    """Assign nodes to 160 bins of exactly 128 nodes, balancing edge load."""
    import heapq

    nbins = NCORES * NW
    order = np.argsort(-deg, kind="stable")
    heap = [(0, b) for b in range(nbins)]
    heapq.heapify(heap)
    load = np.zeros(nbins, np.int64)
    cnt = np.zeros(nbins, np.int32)
    assign = np.empty(N, np.int32)
    for node in order:
        while True:
            _, b = heapq.heappop(heap)
            if cnt[b] < P:
                break
        assign[node] = b
        cnt[b] += 1
        load[b] += deg[node]
        if cnt[b] < P:
            heapq.heappush(heap, (int(load[b]), b))

    target = E // nbins
    bin_nodes = [list(np.where(assign == b)[0]) for b in range(nbins)]
    for _ in range(3000):
        mx = int(load.argmax())
        if load[mx] <= target:
            break
        mn = int(load.argmin())
        da = deg[np.asarray(bin_nodes[mx])]
        db = deg[np.asarray(bin_nodes[mn])]
        want = min(load[mx] - target, target - load[mn])
        best, bi, bj = None, None, None
        for i in range(len(da)):
            d = da[i] - db
            ok = d > 0
            if not ok.any():
                continue
            j = int(np.argmin(np.abs(d - want) + (~ok) * 10**6))
            score = abs(int(d[j]) - want)
            if best is None or score < best:
                best, bi, bj = score, i, j
        if bi is None:
            break
        a, b = bin_nodes[mx][bi], bin_nodes[mn][bj]
        bin_nodes[mx][bi], bin_nodes[mn][bj] = b, a
        load[mx] += deg[b] - deg[a]
        load[mn] += deg[a] - deg[b]
        assign[a], assign[b] = mn, mx
    return assign, int(load.max())


def _host_prep(x, src, dst, graph_ids):
    deg = np.bincount(dst, minlength=N).astype(np.int64)
    assign, maxload = _pack_bins(deg)
    nblk = int(np.ceil(maxload / P))
    nidxw = nblk * P

    # newid: nodes sorted by (bin, old id) -> slot order inside bin
    order_nodes = np.lexsort((np.arange(N), assign))
    newid = np.empty(N, np.int64)
    newid[order_nodes] = np.arange(N)
    old_of_new = order_nodes                    # new id -> old id

    src_n = newid[np.asarray(src).astype(np.int64)]
    dst_n = newid[np.asarray(dst).astype(np.int64)]
    eorder = np.argsort(dst_n, kind="stable")
    src_s = src_n[eorder]
    dst_s = dst_n[eorder]
    win = dst_s // P                            # global window 0..159
    cnt = np.bincount(win, minlength=NCORES * NW)
    assert cnt.max() <= nidxw
    starts = np.zeros(NCORES * NW, np.int64)
    starts[1:] = np.cumsum(cnt)[:-1]

    srcidx = np.zeros((NCORES * NW, nidxw), np.int64)       # pad -> row 0
    dstloc = np.full((NCORES * NW, nidxw), 300.0, np.float32)
    for w in range(NCORES * NW):
        c0, c1 = starts[w], starts[w] + cnt[w]
        srcidx[w, : cnt[w]] = src_s[c0:c1]
        dstloc[w, : cnt[w]] = (dst_s[c0:c1] - w * P).astype(np.float32)

    cols16 = nidxw // 16
    srcidx_d, dstloc_d, selth_d, gmask_d, xcT_d = [], [], [], [], []
    gids = np.asarray(graph_ids).astype(np.int64)
    for c in range(NCORES):
        s16 = np.zeros((128, NW * cols16), np.int16)
        dl = np.full((P, NW * nblk), 300.0, np.float32)
        st = np.zeros((128, NW * nidxw), np.float32)
        for w in range(NW):
            v = srcidx[c * NW + w]
            wrap = v.reshape(cols16, 16).T.astype(np.int16)  # [16, cols16]
            for k in range(8):
                s16[16 * k:16 * (k + 1), w * cols16:(w + 1) * cols16] = wrap
            dv = dstloc[c * NW + w]
            dl[:, w * nblk:(w + 1) * nblk] = dv.reshape(nblk, P).T
            dvi = dv.astype(np.int64)
            ecol = w * nidxw + np.arange(nidxw)
            valid = dvi < P
            st[dvi[valid], ecol[valid]] = 1.0
        srcidx_d.append(np.ascontiguousarray(s16))
        dstloc_d.append(dl.astype(BF))
        selth_d.append(st.astype(BF))

        nodes_c = old_of_new[c * RN:(c + 1) * RN]            # old ids, slot order
        gm = np.zeros((P, NW * G), np.float32)
        for w in range(NW):
            gg = gids[nodes_c[w * P:(w + 1) * P]]
            gm[np.arange(P), w * G + gg] = 1.0
        gmask_d.append(gm.astype(BF))
        xcT_d.append(np.ascontiguousarray(x[nodes_c].T).astype(BF))

    xT = np.ascontiguousarray(x[old_of_new].T).astype(BF)    # [128, N] permuted
    return nblk, xT, srcidx_d, dstloc_d, selth_d, gmask_d, xcT_d


def _blockdiag(a):
    out = np.zeros((HD, H), np.float32)
    for h in range(H):
        out[h * D:(h + 1) * D, h] = a[h]
    return out


# ----------------------------------------------------------------------------
# Device program
# ----------------------------------------------------------------------------
def _build_program(nblk):
    NIDXW = nblk * P
    COLS16 = NIDXW // 16
    # dma_gather is capped at 1024 indices per call (64-descriptor packets)
    chunks = []
    b0 = 0
    while b0 < nblk:
        nb = min(8, nblk - b0)
        chunks.append((b0, nb))
        b0 += nb

    nc = bacc.Bacc(
        "TRN2",
        target_bir_lowering=False,
        debug=False,
        enable_asserts=False,
        num_devices=NCORES,
    )

    xT = nc.dram_tensor("xT", [IN_DIM, N], BF16, kind="ExternalInput")
    xcT = nc.dram_tensor("xcT", [IN_DIM, RN], BF16, kind="ExternalInput")
    Ws, WTs, ALs, ARs = [], [], [], []
    for l, K in enumerate([IN_DIM, HD, HD]):
        Ws.append(nc.dram_tensor(f"W{l}", [K, HD], BF16, kind="ExternalInput"))
        WTs.append(nc.dram_tensor(f"WT{l}", [HD, K], BF16, kind="ExternalInput"))
        ALs.append(nc.dram_tensor(f"albd{l}", [HD, H], BF16, kind="ExternalInput"))
        ARs.append(nc.dram_tensor(f"arbd{l}", [HD, H], BF16, kind="ExternalInput"))
    Wc = nc.dram_tensor("Wc", [HD, C], BF16, kind="ExternalInput")
    bc = nc.dram_tensor("bc_rep", [G, C], F32, kind="ExternalInput")
    srci = nc.dram_tensor("srcidx", [128, NW * COLS16], I16, kind="ExternalInput")
    dstl = nc.dram_tensor("dstloc", [P, NW * nblk], BF16, kind="ExternalInput")
    selth = nc.dram_tensor("selth", [128, NW * NIDXW], BF16, kind="ExternalInput")
    gmk = nc.dram_tensor("gmask", [P, NW * G], BF16, kind="ExternalInput")
    logits = nc.dram_tensor("logits", [G, C], F32, kind="ExternalOutput")

    ztab = [
        nc.dram_tensor("ztab0", [N, ROWT], BF16),
        nc.dram_tensor("ztab1", [N, ROWT], BF16, addr_space="Shared"),
        nc.dram_tensor("ztab2", [N, ROWT], BF16, addr_space="Shared"),
    ]
    zsl = [
        None,
        nc.dram_tensor("zsl1", [RN, ROWT], BF16),
        nc.dram_tensor("zsl2", [RN, ROWT], BF16),
    ]
    hgin = nc.dram_tensor("hgin", [G, HD], F32)
    hgout = nc.dram_tensor("hgout", [G, HD], F32)

    AL = mybir.AluOpType
    ACT_EXP = mybir.ActivationFunctionType.Exp
    ACT_RELU = mybir.ActivationFunctionType.Relu
    ACT_LRELU = mybir.ActivationFunctionType.Lrelu

    with tile.TileContext(nc) as tc:
        with (
            tc.tile_pool(name="const", bufs=1) as constp,
            tc.tile_pool(name="wld", bufs=2) as wldp,
            tc.tile_pool(name="wx", bufs=5) as wxp,
            tc.tile_pool(name="xt", bufs=3) as xtp,
            tc.tile_pool(name="zel", bufs=2) as zelp,
            tc.tile_pool(name="sel", bufs=2) as selp,
            tc.tile_pool(name="mm", bufs=3) as mmp,
            tc.tile_pool(name="small", bufs=2) as smallp,
            tc.tile_pool(name="psmm", bufs=3, space="PSUM") as psmm,
            tc.tile_pool(name="pser", bufs=1, space="PSUM") as pser,
            tc.tile_pool(name="psout", bufs=2, space="PSUM") as psout,
            tc.tile_pool(name="pshg", bufs=1, space="PSUM") as pshg,
        ):
            # ---- constants / resident state ----
            iota_t = constp.tile([P, nblk, P], BF16, tag="iota_t")
            nc.gpsimd.iota(iota_t[:], pattern=[[0, nblk], [1, P]], base=0,
                           channel_multiplier=0,
                           allow_small_or_imprecise_dtypes=True)
            iota_c = constp.tile([P, 1], F32, tag="iota_c")
            nc.gpsimd.iota(iota_c[:], pattern=[[0, 1]], base=0,
                           channel_multiplier=1,
                           allow_small_or_imprecise_dtypes=True)
            ident = constp.tile([P, P], BF16, tag="ident")
            nc.vector.tensor_scalar(
                out=ident[:], in0=iota_t[:, 0, :], scalar1=iota_c[:, 0:1],
                scalar2=None, op0=AL.is_equal,
            )
            neg1 = constp.tile([P, 1], F32, tag="neg1")
            nc.gpsimd.memset(neg1[:], -1.0)
            srci_sb = constp.tile([128, NW * COLS16], I16, tag="srci")
            nc.sync.dma_start(srci_sb[:], srci[:, :])
            dstl_sb = constp.tile([P, NW * nblk], BF16, tag="dstl")
            nc.sync.dma_start(dstl_sb[:], dstl[:, :])
            gmk_sb = constp.tile([P, NW * G], BF16, tag="gmk")
            nc.sync.dma_start(gmk_sb[:], gmk[:, :])
            xc_sb = constp.tile([IN_DIM, RN], BF16, tag="xc")
            nc.sync.dma_start(xc_sb[:], xcT[:, :])
            h_all = constp.tile([P, NW, HD], F32, tag="h_all")
            hb_all = constp.tile([P, NW, HD], BF16, tag="hb_all")
            herb = constp.tile([P, NW, H], BF16, tag="herb")
            hg_acc = constp.tile([G, HD], F32, tag="hg_acc")
            nc.gpsimd.memset(hg_acc[:], 0.0)

            def build_wext(l, K):
                kch = K // P
                och = HD // P
                W_sb, WT_sb, al_sb, ar_sb = [], [], [], []
                for k in range(kch):
                    t = wldp.tile([P, HD], BF16, tag="wld")
                    nc.sync.dma_start(t[:], Ws[l][k * P:(k + 1) * P, :])
                    W_sb.append(t)
                for oc in range(och):
                    t = wldp.tile([P, K], BF16, tag="wtld")
                    nc.sync.dma_start(t[:], WTs[l][oc * P:(oc + 1) * P, :])
                    WT_sb.append(t)
                    ta = wldp.tile([P, H], BF16, tag="alld")
                    nc.sync.dma_start(ta[:], ALs[l][oc * P:(oc + 1) * P, :])
                    al_sb.append(ta)
                    tr = wldp.tile([P, H], BF16, tag="arld")
                    nc.sync.dma_start(tr[:], ARs[l][oc * P:(oc + 1) * P, :])
                    ar_sb.append(tr)
                wext = []
                for k in range(kch):
                    wx = wxp.tile([P, ROWT], BF16, tag="wext")
                    nc.vector.tensor_copy(wx[:, 0:HD], W_sb[k][:])
                    for dstcol, bd in ((HD, al_sb), (HD + H, ar_sb)):
                        ps = psmm.tile([P, H], F32, tag="mm")
                        for oc in range(och):
                            nc.tensor.matmul(
                                ps[:],
                                lhsT=WT_sb[oc][:, k * P:(k + 1) * P],
                                rhs=bd[oc][:],
                                start=(oc == 0),
                                stop=(oc == och - 1),
                            )
                        nc.vector.tensor_copy(wx[:, dstcol:dstcol + H], ps[:])
                    # zero the pad so the table holds no stray NaN/Inf
                    nc.gpsimd.memset(wx[:, HD + 2 * H:], 0.0)
                    wext.append(wx)
                return wext

            def l0_table(wext0):
                CH = 8  # x tiles per DMA
                for t8 in range(N // (P * CH)):
                    xt8 = xtp.tile([P, CH * P], BF16, tag="xt")
                    nc.sync.dma_start(xt8[:], xT[:, t8 * CH * P:(t8 + 1) * CH * P])
                    for t in range(CH):
                        zp = psmm.tile([P, ROWT], F32, tag="mm")
                        nc.tensor.matmul(
                            zp[:], lhsT=xt8[:, t * P:(t + 1) * P],
                            rhs=wext0[0][:], start=True, stop=True,
                        )
                        zs = mmp.tile([P, ROWT], BF16, tag="zs")
                        nc.vector.tensor_copy(zs[:], zp[:])
                        row = (t8 * CH + t) * P
                        nc.sync.dma_start(ztab[0][row:row + P, :], zs[:])
                # own-slice er (tiny N=4 matmuls; avoids core-dependent reads)
                for w in range(NW):
                    ep = psmm.tile([P, H], F32, tag="mm")
                    nc.tensor.matmul(
                        ep[:], lhsT=xc_sb[:, w * P:(w + 1) * P],
                        rhs=wext0[0][:, HD + H:HD + 2 * H],
                        start=True, stop=True,
                    )
                    nc.vector.tensor_copy(herb[:, w, :], ep[:])

            def slice_window(l, wext, w):
                """z|el|er for own window w of layer l (l>=1) -> zsl[l]."""
                hts = []
                for c2 in range(2):
                    tp = psmm.tile([P, P], BF16, tag="mm")
                    nc.tensor.transpose(
                        tp[:], hb_all[:, w, c2 * P:(c2 + 1) * P], ident[:]
                    )
                    ht = mmp.tile([P, P], BF16, tag="ht")
                    nc.vector.tensor_copy(ht[:], tp[:])
                    hts.append(ht)
                zp = psmm.tile([P, ROWT], F32, tag="mm")
                for c2 in range(2):
                    nc.tensor.matmul(
                        zp[:], lhsT=hts[c2][:], rhs=wext[c2][:],
                        start=(c2 == 0), stop=(c2 == 1),
                    )
                zs = mmp.tile([P, ROWT], BF16, tag="zs")
                nc.vector.tensor_copy(zs[:], zp[:])
                nc.vector.tensor_copy(herb[:, w, :], zp[:, HD + H:HD + 2 * H])
                nc.sync.dma_start(zsl[l][w * P:(w + 1) * P, :], zs[:])

            def edge_window(l, w):
                # gather z_ext[src] rows for this window
                zel = zelp.tile([P, nblk, ROWT], BF16, tag="zel")
                for b0, nb in chunks:
                    nc.gpsimd.dma_gather(
                        out_ap=zel[:, b0:b0 + nb, :],
                        in_ap=ztab[l][:, :],
                        idxs_ap=srci_sb[:, w * COLS16 + b0 * 8:
                                        w * COLS16 + (b0 + nb) * 8],
                        num_idxs=nb * P,
                        num_idxs_reg=nb * P,
                        elem_size=ROWT,
                    )
                # selt[d, e] = one-hot of dst (transposed layout; host-built)
                selt = selp.tile([P, NIDXW], BF16, tag="selt")
                nc.sync.dma_start(selt[:], selth[:, w * NIDXW:(w + 1) * NIDXW])
                # sel[e, d] = one-hot of dst (edge-major layout)
                sel = selp.tile([P, nblk, P], BF16, tag="sel")
                nc.vector.tensor_tensor(
                    out=sel[:], in0=iota_t[:],
                    in1=dstl_sb[:, w * nblk:(w + 1) * nblk]
                    .to_broadcast([P, nblk, P]),
                    op=AL.is_equal,
                )
                # er expansion to edges: per block  erE = selt_b.T @ er_w
                erp = pser.tile([P, nblk * H], F32, tag="er")
                for b in range(nblk):
                    nc.tensor.matmul(
                        erp[:, b * H:(b + 1) * H],
                        lhsT=selt[:, b * P:(b + 1) * P],
                        rhs=herb[:, w, :],
                        start=True, stop=True,
                    )
                erb = mmp.tile([P, nblk, H], BF16, tag="erb")
                nc.vector.tensor_copy(erb[:], erp[:])
                # e = leaky_relu(el + er);  ex = exp(e)  (into zel's el slot)
                eall = mmp.tile([P, nblk, H], F32, tag="eall")
                nc.vector.tensor_tensor(
                    out=eall[:], in0=zel[:, :, HD:HD + H], in1=erb[:], op=AL.add
                )
                et = mmp.tile([P, nblk, H], F32, tag="et")
                nc.vector.tensor_scalar_mul(et[:], eall[:], NEG_SLOPE)
                nc.vector.tensor_tensor(out=eall[:], in0=eall[:], in1=et[:],
                                        op=AL.max)
                nc.scalar.activation(zel[:, :, HD:HD + H], eall[:], ACT_EXP)
                # msg = z * ex  (in place)
                zb = zel[:, :, 0:HD].rearrange("p b (h d) -> p b h d", h=H)
                nc.vector.tensor_tensor(
                    out=zb, in0=zb,
                    in1=zel[:, :, HD:HD + H].to_broadcast([P, nblk, H, D]),
                    op=AL.mult,
                )
                # scatter-add [msg | sum_ex] via one-hot matmuls
                outp = psout.tile([P, HD + H], F32, tag="out")
                for b in range(nblk):
                    nc.tensor.matmul(
                        outp[:],
                        lhsT=sel[:, b, :],
                        rhs=zel[:, b, 0:HD + H],
                        start=(b == 0),
                        stop=(b == nblk - 1),
                    )
                # normalize + residual + activation(s)
                rec = mmp.tile([P, H], F32, tag="rec")
                nc.vector.reciprocal(rec[:], outp[:, HD:HD + H])
                agg = mmp.tile([P, HD], F32, tag="agg")
                nc.vector.tensor_tensor(
                    out=agg[:].rearrange("p (h d) -> p h d", h=H),
                    in0=outp[:, 0:HD].rearrange("p (h d) -> p h d", h=H),
                    in1=rec[:].to_broadcast([P, H, D]),
                    op=AL.mult,
                )
                # elu(x) = Relu(x) + Exp(-Relu(-x)) - 1, mostly on the ACT
                # engine (tensor_scalar on DVE measures ~3.7x slower than
                # tensor_tensor, so the -1 uses a broadcast const add).
                mn = mmp.tile([P, HD], F32, tag="emn")
                ex = mmp.tile([P, HD], F32, tag="eex")
                if l == 0:
                    # h = elu(agg)
                    nc.scalar.activation(mn[:], agg[:], ACT_RELU, scale=-1.0)
                    nc.scalar.activation(ex[:], mn[:], ACT_EXP, scale=-1.0)
                    nc.scalar.activation(mn[:], agg[:], ACT_RELU)
                    nc.vector.tensor_tensor(out=ex[:], in0=ex[:], in1=mn[:],
                                            op=AL.add)
                    nc.vector.tensor_tensor(
                        out=h_all[:, w, :], in0=ex[:],
                        in1=neg1[:].to_broadcast([P, HD]), op=AL.add)
                else:
                    # h = elu(elu(agg + h))  (fused double elu):
                    #   n = Relu(-x); e = Exp(Exp(-n) - 1); h = Relu(x) + e - 1
                    nc.vector.tensor_tensor(out=agg[:], in0=agg[:],
                                            in1=h_all[:, w, :], op=AL.add)
                    nc.scalar.activation(mn[:], agg[:], ACT_RELU, scale=-1.0)
                    nc.scalar.activation(ex[:], mn[:], ACT_EXP, scale=-1.0)
                    nc.scalar.activation(ex[:], ex[:], ACT_EXP, bias=neg1[:])
                    nc.scalar.activation(mn[:], agg[:], ACT_RELU)
                    nc.vector.tensor_tensor(out=ex[:], in0=ex[:], in1=mn[:],
                                            op=AL.add)
                    nc.vector.tensor_tensor(
                        out=h_all[:, w, :], in0=ex[:],
                        in1=neg1[:].to_broadcast([P, HD]), op=AL.add)
                nc.vector.tensor_copy(hb_all[:, w, :], h_all[:, w, :])

            def pool_window(w):
                gp = pshg.tile([G, HD], F32, tag="hg")
                nc.tensor.matmul(
                    gp[:],
                    lhsT=gmk_sb[:, w * G:(w + 1) * G],
                    rhs=hb_all[:, w, :],
                    start=True, stop=True,
                )
                nc.vector.tensor_tensor(out=hg_acc[:], in0=hg_acc[:],
                                        in1=gp[:], op=AL.add)

            # ---- program ----
            wext = [build_wext(0, IN_DIM), build_wext(1, HD), build_wext(2, HD)]
            l0_table(wext[0])
            for l in range(3):
                for w in range(NW):
                    edge_window(l, w)
                    if l < 2:
                        slice_window(l + 1, wext[l + 1], w)
                    else:
                        pool_window(w)
                if l < 2:
                    nc.gpsimd.collective_compute(
                        "AllGather", AL.bypass,
                        replica_groups=[list(range(NCORES))],
                        ins=[zsl[l + 1][:, :]],
                        outs=[ztab[l + 1][:, :]],
                    )

            # ---- epilogue: AllReduce graph sums -> mean -> elu -> @Wc+bc ----
            nc.sync.dma_start(hgin[:, :], hg_acc[:])
            nc.gpsimd.collective_compute(
                "AllReduce", AL.add,
                replica_groups=[list(range(NCORES))],
                ins=[hgin[:, :]], outs=[hgout[:, :]],
            )
            hgr = smallp.tile([G, HD], F32, tag="hgr")
            nc.sync.dma_start(hgr[:], hgout[:, :])
            nc.vector.tensor_scalar_mul(hgr[:], hgr[:], 1.0 / NPG)
            mn = smallp.tile([G, HD], F32, tag="fmn")
            exx = smallp.tile([G, HD], F32, tag="fex")
            nc.vector.tensor_scalar_min(mn[:], hgr[:], 0.0)
            nc.scalar.activation(exx[:], mn[:], ACT_EXP)
            nc.vector.tensor_scalar_add(exx[:], exx[:], -1.0)
            nc.vector.tensor_scalar_max(mn[:], hgr[:], 0.0)
            nc.vector.tensor_tensor(out=hgr[:], in0=exx[:], in1=mn[:], op=AL.add)
            hgb = smallp.tile([G, HD], BF16, tag="hgb")
            nc.vector.tensor_copy(hgb[:], hgr[:])

            wc_sb, hgts = [], []
            for c2 in range(2):
                t = smallp.tile([P, C], BF16, tag="wc")
                nc.sync.dma_start(t[:], Wc[c2 * P:(c2 + 1) * P, :])
                wc_sb.append(t)
                tp = psmm.tile([P, G], BF16, tag="mm")
                nc.tensor.transpose(
                    tp[:], hgb[:, c2 * P:(c2 + 1) * P], ident[:G, :G]
                )
                hgt = smallp.tile([P, G], BF16, tag="hgt")
                nc.vector.tensor_copy(hgt[:], tp[:])
                hgts.append(hgt)
            lg = psmm.tile([G, C], F32, tag="mm")
            for c2 in range(2):
                nc.tensor.matmul(
                    lg[:], lhsT=hgts[c2][:], rhs=wc_sb[c2][:],
                    start=(c2 == 0), stop=(c2 == 1),
                )
            bc_sb = smallp.tile([G, C], F32, tag="bc")
            nc.sync.dma_start(bc_sb[:], bc[:, :])
            lg_sb = smallp.tile([G, C], F32, tag="lg")
            nc.vector.tensor_tensor(out=lg_sb[:], in0=lg[:], in1=bc_sb[:],
                                    op=AL.add)
            nc.sync.dma_start(logits[:, :], lg_sb[:])

    nc.compile()
    return nc


def _get_program(nblk):
    if nblk not in _CACHE:
        _CACHE[nblk] = _build_program(nblk)
    return _CACHE[nblk]


# ----------------------------------------------------------------------------
# Entry point
# ----------------------------------------------------------------------------
def kernel(x, src, dst, graph_ids, W0, al0, ar0, W1, al1, ar1, W2, al2, ar2, Wc, bc):
    global LAST_EXEC_NS, LAST_RESULTS
    x = np.ascontiguousarray(np.asarray(x, np.float32))
    src = np.asarray(src).astype(np.int64)
    dst = np.asarray(dst).astype(np.int64)
    graph_ids = np.asarray(graph_ids).astype(np.int64)

    nblk, xT, srcidx_d, dstloc_d, selth_d, gmask_d, xcT_d = _host_prep(
        x, src, dst, graph_ids)
    nc = _get_program(nblk)

    Wl = [np.asarray(W0, np.float32), np.asarray(W1, np.float32),
          np.asarray(W2, np.float32)]
    als = [al0, al1, al2]
    ars = [ar0, ar1, ar2]
    common = {
        "xT": xT,
        "Wc": np.asarray(Wc, np.float32).astype(BF),
        "bc_rep": np.tile(np.asarray(bc, np.float32)[None, :], (G, 1)),
    }
    for l in range(3):
        common[f"W{l}"] = Wl[l].astype(BF)
        common[f"WT{l}"] = np.ascontiguousarray(Wl[l].T).astype(BF)
        common[f"albd{l}"] = _blockdiag(np.asarray(als[l], np.float32)).astype(BF)
        common[f"arbd{l}"] = _blockdiag(np.asarray(ars[l], np.float32)).astype(BF)

    in_maps = []
    for c in range(NCORES):
        m = dict(common)
        m["xcT"] = xcT_d[c]
        m["srcidx"] = srcidx_d[c]
        m["dstloc"] = dstloc_d[c]
        m["selth"] = selth_d[c]
        m["gmask"] = gmask_d[c]
        in_maps.append(m)

    import os
    if os.environ.get("KERNEL_SIM"):
        from concourse.bass_interp import MultiCoreSim

        sim = MultiCoreSim(nc, num_cores=NCORES)
        for c, core_sim in sim.cores.items():
            for k, v in in_maps[c].items():
                core_sim.tensor(k)[:] = v
        sim.simulate()
        LAST_EXEC_NS = None
        return np.asarray(sim.cores[0].tensor("logits")).astype(np.float32)

    if TRACE:
        _install_ntff_hook_shim()
    res = run_bass_kernel_spmd(nc, in_maps, list(range(NCORES)), trace=TRACE)
    LAST_EXEC_NS = res.exec_time_ns
    LAST_RESULTS = res
    return res.results[0]["logits"].astype(np.float32)


# revision 26
# speedup vs baseline: 1.3623x; 1.3623x over previous
"""3-layer GAT + per-graph mean-pool + linear head, distributed over 8 NeuronCores.

Strategy (edge-parallel, dst-balanced, bf16):
  * Host: bin-pack nodes into 160 (core, window) bins of 128 nodes each so
    every window owns exactly ~2048 incoming edges (nblk=16 blocks of 128);
    remap node ids to (core, window, slot) order and sort edges by dst.
  * Per layer each core computes z|el|er for its own 2560 nodes (one matmul
    per window; attention logits folded in via Wel = W @ albd) into a bf16
    node table with 384-element rows [z(256)|el(4)|er(4)|pad]; the table is
    AllGather'd (Shared scratch) for layers 1-2; layer 0's table is computed
    fully replicated from the (replicated) input features.
  * Edge phase per window: two 1024-row dma_gather calls pull z_ext[src]
    rows (768 B each); er[dst] stays on-core (bf16 [128,4] per window) and is
    expanded to edges with one-hot matmuls whose selector selt is built with
    a gpsimd partition_broadcast of the host-provided dst row + one is_eq.
    ex = exp(leaky_relu(el+er)) is batched per window; messages are scaled in
    place and scatter-added with one-hot bf16 matmuls into PSUM [out|sum_ex].
    Softmax max-shift is dropped (shift-invariant; logits cannot overflow).
  * Pooling: per-window one-hot matmul accumulates per-graph sums over all
    64 graphs; a [64,256] AllReduce combines cores; every core computes the
    full logits and the host takes core 0's.
"""

import sys

import numpy as np

sys.path.insert(0, "/opt/trn_rl_repo")

import ml_dtypes

import concourse.bass as bass
import concourse.bacc as bacc
import concourse.mybir as mybir
import concourse.tile as tile
from concourse.bass_utils import run_bass_kernel_spmd

# Problem shape (hardcoded per contest rules).
N, E, G = 20480, 327680, 64
IN_DIM, H, D, C = 128, 4, 64, 10
HD = H * D            # 256
ROWT = 384            # bf16 table row: z(256) | el(4) | er(4) | pad(120)
NCORES = 8
RN = N // NCORES      # 2560 dst nodes per core
P = 128
NW = RN // P          # 20 windows per core
NPG = N // G          # 320 nodes per graph
NEG_SLOPE = 0.2
F32 = mybir.dt.float32
BF16 = mybir.dt.bfloat16
I16 = mybir.dt.int16
I32 = mybir.dt.int32
BF = ml_dtypes.bfloat16

TRACE = False         # set by test.py to capture HW profile
LAST_EXEC_NS = None
LAST_RESULTS = None

_CACHE = {}


def _install_ntff_hook_shim():
    """This image's ``antenv`` lacks ``axon_hooks``; provide the thin ctypes
    shim around libaxon_pjrt.so so run_bass_kernel_spmd(trace=True) works."""
    try:
        import antenv.axon_hooks  # noqa: F401
        return
    except ImportError:
        pass
    import contextlib
    import ctypes
    import types

    so_path = "/opt/axon/libaxon_pjrt.so"
    try:
        lib = ctypes.CDLL(so_path)
    except OSError:
        return
    if not hasattr(lib, "axon_start_nrt_profile"):
        return
    lib.axon_start_nrt_profile.argtypes = [ctypes.POINTER(ctypes.c_int64), ctypes.c_size_t]
    lib.axon_start_nrt_profile.restype = ctypes.c_int64
    lib.axon_stop_nrt_profile.argtypes = [ctypes.c_char_p]
    lib.axon_stop_nrt_profile.restype = ctypes.c_int64

    @contextlib.contextmanager
    def _hook(output_dir, device_ids):
        import jax

        jax.devices()
        if device_ids:
            ids = (ctypes.c_int64 * len(device_ids))(*device_ids)
            rc = lib.axon_start_nrt_profile(ids, len(device_ids))
        else:
            rc = lib.axon_start_nrt_profile(None, 0)
        if rc != 0:
            raise RuntimeError(f"axon_start_nrt_profile rc={rc}")
        try:
            yield
        finally:
            n = lib.axon_stop_nrt_profile(str(output_dir).encode())
            print(f"ntff profile: {n} file(s) written to {output_dir}")

    mod = types.ModuleType("antenv.axon_hooks")
    mod.get_axon_ntff_profile_hook = lambda: _hook
    mod.set_axon_ntff_profile_hook = lambda h: None
    sys.modules["antenv.axon_hooks"] = mod


# ----------------------------------------------------------------------------
# Host-side preprocessing (sharding / layout only)
# ----------------------------------------------------------------------------
def _pack_bins(deg):
    """Assign nodes to 160 bins of exactly 128 nodes, balancing edge load."""
    import heapq

    nbins = NCORES * NW
    order = np.argsort(-deg, kind="stable")
    heap = [(0, b) for b in range(nbins)]
    heapq.heapify(heap)
    load = np.zeros(nbins, np.int64)
    cnt = np.zeros(nbins, np.int32)
    assign = np.empty(N, np.int32)
    for node in order:
        while True:
            _, b = heapq.heappop(heap)
            if cnt[b] < P:
                break
        assign[node] = b
        cnt[b] += 1
        load[b] += deg[node]
        if cnt[b] < P:
            heapq.heappush(heap, (int(load[b]), b))

    target = E // nbins
    bin_nodes = [list(np.where(assign == b)[0]) for b in range(nbins)]
    for _ in range(3000):
        mx = int(load.argmax())
        if load[mx] <= target:
            break
        mn = int(load.argmin())
        da = deg[np.asarray(bin_nodes[mx])]
        db = deg[np.asarray(bin_nodes[mn])]
        want = min(load[mx] - target, target - load[mn])
        best, bi, bj = None, None, None
        for i in range(len(da)):
            d = da[i] - db
            ok = d > 0
            if not ok.any():
                continue
            j = int(np.argmin(np.abs(d - want) + (~ok) * 10**6))
            score = abs(int(d[j]) - want)
            if best is None or score < best:
                best, bi, bj = score, i, j
        if bi is None:
            break
        a, b = bin_nodes[mx][bi], bin_nodes[mn][bj]
        bin_nodes[mx][bi], bin_nodes[mn][bj] = b, a
        load[mx] += deg[b] - deg[a]
        load[mn] += deg[a] - deg[b]
        assign[a], assign[b] = mn, mx
    return assign, int(load.max())


def _host_prep(x, src, dst, graph_ids):
    deg = np.bincount(dst, minlength=N).astype(np.int64)
    assign, maxload = _pack_bins(deg)
    nblk = int(np.ceil(maxload / P))
    nidxw = nblk * P

    # newid: nodes sorted by (bin, old id) -> slot order inside bin
    order_nodes = np.lexsort((np.arange(N), assign))
    newid = np.empty(N, np.int64)
    newid[order_nodes] = np.arange(N)
    old_of_new = order_nodes                    # new id -> old id

    src_n = newid[np.asarray(src).astype(np.int64)]
    dst_n = newid[np.asarray(dst).astype(np.int64)]
    eorder = np.argsort(dst_n, kind="stable")
    src_s = src_n[eorder]
    dst_s = dst_n[eorder]
    win = dst_s // P                            # global window 0..159
    cnt = np.bincount(win, minlength=NCORES * NW)
    assert cnt.max() <= nidxw
    starts = np.zeros(NCORES * NW, np.int64)
    starts[1:] = np.cumsum(cnt)[:-1]

    srcidx = np.zeros((NCORES * NW, nidxw), np.int64)       # pad -> row 0
    dstloc = np.full((NCORES * NW, nidxw), 300.0, np.float32)
    for w in range(NCORES * NW):
        c0, c1 = starts[w], starts[w] + cnt[w]
        srcidx[w, : cnt[w]] = src_s[c0:c1]
        dstloc[w, : cnt[w]] = (dst_s[c0:c1] - w * P).astype(np.float32)

    cols16 = nidxw // 16
    srcidx_d, dstloc_d, selth_d, gmask_d, xcT_d = [], [], [], [], []
    gids = np.asarray(graph_ids).astype(np.int64)
    for c in range(NCORES):
        s16 = np.zeros((128, NW * cols16), np.int16)
        dl = np.full((P, NW * nblk), 300.0, np.float32)
        st = np.zeros((128, NW * nidxw), np.float32)
        for w in range(NW):
            v = srcidx[c * NW + w]
            wrap = v.reshape(cols16, 16).T.astype(np.int16)  # [16, cols16]
            for k in range(8):
                s16[16 * k:16 * (k + 1), w * cols16:(w + 1) * cols16] = wrap
            dv = dstloc[c * NW + w]
            dl[:, w * nblk:(w + 1) * nblk] = dv.reshape(nblk, P).T
            dvi = dv.astype(np.int64)
            ecol = w * nidxw + np.arange(nidxw)
            valid = dvi < P
            st[dvi[valid], ecol[valid]] = 1.0
        srcidx_d.append(np.ascontiguousarray(s16))
        dstloc_d.append(dl.astype(BF))
        selth_d.append(st.astype(BF))

        nodes_c = old_of_new[c * RN:(c + 1) * RN]            # old ids, slot order
        gm = np.zeros((P, NW * G), np.float32)
        for w in range(NW):
            gg = gids[nodes_c[w * P:(w + 1) * P]]
            gm[np.arange(P), w * G + gg] = 1.0
        gmask_d.append(gm.astype(BF))
        xcT_d.append(np.ascontiguousarray(x[nodes_c].T).astype(BF))

    xT = np.ascontiguousarray(x[old_of_new].T).astype(BF)    # [128, N] permuted
    return nblk, xT, srcidx_d, dstloc_d, selth_d, gmask_d, xcT_d


def _blockdiag(a):
    out = np.zeros((HD, H), np.float32)
    for h in range(H):
        out[h * D:(h + 1) * D, h] = a[h]
    return out


# ----------------------------------------------------------------------------
# Device program
# ----------------------------------------------------------------------------
def _build_program(nblk):
    NIDXW = nblk * P
    COLS16 = NIDXW // 16
    # dma_gather is capped at 1024 indices per call (64-descriptor packets)
    chunks = []
    b0 = 0
    while b0 < nblk:
        nb = min(8, nblk - b0)
        chunks.append((b0, nb))
        b0 += nb

    nc = bacc.Bacc(
        "TRN2",
        target_bir_lowering=False,
        debug=False,
        enable_asserts=False,
        num_devices=NCORES,
    )

    xT = nc.dram_tensor("xT", [IN_DIM, N], BF16, kind="ExternalInput")
    xcT = nc.dram_tensor("xcT", [IN_DIM, RN], BF16, kind="ExternalInput")
    Ws, WTs, ALs, ARs = [], [], [], []
    for l, K in enumerate([IN_DIM, HD, HD]):
        Ws.append(nc.dram_tensor(f"W{l}", [K, HD], BF16, kind="ExternalInput"))
        WTs.append(nc.dram_tensor(f"WT{l}", [HD, K], BF16, kind="ExternalInput"))
        ALs.append(nc.dram_tensor(f"albd{l}", [HD, H], BF16, kind="ExternalInput"))
        ARs.append(nc.dram_tensor(f"arbd{l}", [HD, H], BF16, kind="ExternalInput"))
    Wc = nc.dram_tensor("Wc", [HD, C], BF16, kind="ExternalInput")
    bc = nc.dram_tensor("bc_rep", [G, C], F32, kind="ExternalInput")
    srci = nc.dram_tensor("srcidx", [128, NW * COLS16], I16, kind="ExternalInput")
    dstl = nc.dram_tensor("dstloc", [P, NW * nblk], BF16, kind="ExternalInput")
    selth = nc.dram_tensor("selth", [128, NW * NIDXW], BF16, kind="ExternalInput")
    gmk = nc.dram_tensor("gmask", [P, NW * G], BF16, kind="ExternalInput")
    logits = nc.dram_tensor("logits", [G, C], F32, kind="ExternalOutput")

    ztab = [
        nc.dram_tensor("ztab0", [N, ROWT], BF16),
        nc.dram_tensor("ztab1", [N, ROWT], BF16, addr_space="Shared"),
        nc.dram_tensor("ztab2", [N, ROWT], BF16, addr_space="Shared"),
    ]
    zsl = [
        None,
        nc.dram_tensor("zsl1", [RN, ROWT], BF16),
        nc.dram_tensor("zsl2", [RN, ROWT], BF16),
    ]
    hgin = nc.dram_tensor("hgin", [G, HD], F32)
    hgout = nc.dram_tensor("hgout", [G, HD], F32)

    AL = mybir.AluOpType
    ACT_EXP = mybir.ActivationFunctionType.Exp
    ACT_RELU = mybir.ActivationFunctionType.Relu
    ACT_LRELU = mybir.ActivationFunctionType.Lrelu

    with tile.TileContext(nc) as tc:
        with (
            tc.tile_pool(name="const", bufs=1) as constp,
            tc.tile_pool(name="wld", bufs=2) as wldp,
            tc.tile_pool(name="wx", bufs=5) as wxp,
            tc.tile_pool(name="xt", bufs=3) as xtp,
            tc.tile_pool(name="zel", bufs=4) as zelp,
            tc.tile_pool(name="sel", bufs=3) as selp,
            tc.tile_pool(name="mm", bufs=4) as mmp,
            tc.tile_pool(name="small", bufs=2) as smallp,
            tc.tile_pool(name="psmm", bufs=3, space="PSUM") as psmm,
            tc.tile_pool(name="pser", bufs=2, space="PSUM") as pser,
            tc.tile_pool(name="psout", bufs=2, space="PSUM") as psout,
            tc.tile_pool(name="pshg", bufs=1, space="PSUM") as pshg,
        ):
            # ---- constants / resident state ----
            iota_t = constp.tile([P, nblk, P], BF16, tag="iota_t")
            nc.gpsimd.iota(iota_t[:], pattern=[[0, nblk], [1, P]], base=0,
                           channel_multiplier=0,
                           allow_small_or_imprecise_dtypes=True)
            iota_c = constp.tile([P, 1], F32, tag="iota_c")
            nc.gpsimd.iota(iota_c[:], pattern=[[0, 1]], base=0,
                           channel_multiplier=1,
                           allow_small_or_imprecise_dtypes=True)
            ident = constp.tile([P, P], BF16, tag="ident")
            nc.vector.tensor_scalar(
                out=ident[:], in0=iota_t[:, 0, :], scalar1=iota_c[:, 0:1],
                scalar2=None, op0=AL.is_equal,
            )
            neg1 = constp.tile([P, 1], F32, tag="neg1")
            nc.gpsimd.memset(neg1[:], -1.0)
            srci_sb = constp.tile([128, NW * COLS16], I16, tag="srci")
            nc.sync.dma_start(srci_sb[:], srci[:, :])
            dstl_sb = constp.tile([P, NW * nblk], BF16, tag="dstl")
            nc.sync.dma_start(dstl_sb[:], dstl[:, :])
            gmk_sb = constp.tile([P, NW * G], BF16, tag="gmk")
            nc.sync.dma_start(gmk_sb[:], gmk[:, :])
            xc_sb = constp.tile([IN_DIM, RN], BF16, tag="xc")
            nc.sync.dma_start(xc_sb[:], xcT[:, :])
            h_all = constp.tile([P, NW, HD], F32, tag="h_all")
            hb_all = constp.tile([P, NW, HD], BF16, tag="hb_all")
            herb = constp.tile([P, NW, H], BF16, tag="herb")
            hg_acc = constp.tile([G, HD], F32, tag="hg_acc")
            nc.gpsimd.memset(hg_acc[:], 0.0)

            def build_wext(l, K):
                kch = K // P
                och = HD // P
                W_sb, WT_sb, al_sb, ar_sb = [], [], [], []
                for k in range(kch):
                    t = wldp.tile([P, HD], BF16, tag="wld")
                    nc.sync.dma_start(t[:], Ws[l][k * P:(k + 1) * P, :])
                    W_sb.append(t)
                for oc in range(och):
                    t = wldp.tile([P, K], BF16, tag="wtld")
                    nc.sync.dma_start(t[:], WTs[l][oc * P:(oc + 1) * P, :])
                    WT_sb.append(t)
                    ta = wldp.tile([P, H], BF16, tag="alld")
                    nc.sync.dma_start(ta[:], ALs[l][oc * P:(oc + 1) * P, :])
                    al_sb.append(ta)
                    tr = wldp.tile([P, H], BF16, tag="arld")
                    nc.sync.dma_start(tr[:], ARs[l][oc * P:(oc + 1) * P, :])
                    ar_sb.append(tr)
                wext = []
                for k in range(kch):
                    wx = wxp.tile([P, ROWT], BF16, tag="wext")
                    nc.vector.tensor_copy(wx[:, 0:HD], W_sb[k][:])
                    for dstcol, bd in ((HD, al_sb), (HD + H, ar_sb)):
                        ps = psmm.tile([P, H], F32, tag="mm")
                        for oc in range(och):
                            nc.tensor.matmul(
                                ps[:],
                                lhsT=WT_sb[oc][:, k * P:(k + 1) * P],
                                rhs=bd[oc][:],
                                start=(oc == 0),
                                stop=(oc == och - 1),
                            )
                        nc.vector.tensor_copy(wx[:, dstcol:dstcol + H], ps[:])
                    # zero the pad so the table holds no stray NaN/Inf
                    nc.gpsimd.memset(wx[:, HD + 2 * H:], 0.0)
                    wext.append(wx)
                return wext

            def l0_table(wext0):
                CH = 8  # x tiles per input DMA; one batched output DMA per CH
                for t8 in range(N // (P * CH)):
                    xt8 = xtp.tile([P, CH * P], BF16, tag="xt")
                    nc.sync.dma_start(xt8[:], xT[:, t8 * CH * P:(t8 + 1) * CH * P])
                    zs8 = xtp.tile([P, CH, ROWT], BF16, tag="zs8")
                    for t in range(CH):
                        zp = psmm.tile([P, ROWT], F32, tag="mm")
                        nc.tensor.matmul(
                            zp[:], lhsT=xt8[:, t * P:(t + 1) * P],
                            rhs=wext0[0][:], start=True, stop=True,
                        )
                        nc.vector.tensor_copy(zs8[:, t, :], zp[:])
                    row = t8 * CH * P
                    nc.sync.dma_start(
                        ztab[0][row:row + CH * P, :]
                        .rearrange("(c p) f -> p c f", c=CH),
                        zs8[:],
                    )
                # own-slice er (tiny N=4 matmuls; avoids core-dependent reads)
                for w in range(NW):
                    ep = psmm.tile([P, H], F32, tag="mm")
                    nc.tensor.matmul(
                        ep[:], lhsT=xc_sb[:, w * P:(w + 1) * P],
                        rhs=wext0[0][:, HD + H:HD + 2 * H],
                        start=True, stop=True,
                    )
                    nc.vector.tensor_copy(herb[:, w, :], ep[:])

            def slice_window(l, wext, w):
                """z|el|er for own window w of layer l (l>=1) -> zsl[l]."""
                hts = []
                for c2 in range(2):
                    tp = psmm.tile([P, P], BF16, tag="mm")
                    nc.tensor.transpose(
                        tp[:], hb_all[:, w, c2 * P:(c2 + 1) * P], ident[:]
                    )
                    ht = mmp.tile([P, P], BF16, tag="ht")
                    nc.vector.tensor_copy(ht[:], tp[:])
                    hts.append(ht)
                zp = psmm.tile([P, ROWT], F32, tag="mm")
                for c2 in range(2):
                    nc.tensor.matmul(
                        zp[:], lhsT=hts[c2][:], rhs=wext[c2][:],
                        start=(c2 == 0), stop=(c2 == 1),
                    )
                zs = mmp.tile([P, ROWT], BF16, tag="zs")
                nc.vector.tensor_copy(zs[:], zp[:])
                nc.vector.tensor_copy(herb[:, w, :], zp[:, HD + H:HD + 2 * H])
                nc.sync.dma_start(zsl[l][w * P:(w + 1) * P, :], zs[:])

            def edge_window(l, w):
                # gather z_ext[src] rows for this window
                zel = zelp.tile([P, nblk, ROWT], BF16, tag="zel")
                for b0, nb in chunks:
                    nc.gpsimd.dma_gather(
                        out_ap=zel[:, b0:b0 + nb, :],
                        in_ap=ztab[l][:, :],
                        idxs_ap=srci_sb[:, w * COLS16 + b0 * 8:
                                        w * COLS16 + (b0 + nb) * 8],
                        num_idxs=nb * P,
                        num_idxs_reg=nb * P,
                        elem_size=ROWT,
                    )
                # selt[d, e] = one-hot of dst (transposed layout; host-built)
                selt = selp.tile([P, NIDXW], BF16, tag="selt")
                nc.sync.dma_start(selt[:], selth[:, w * NIDXW:(w + 1) * NIDXW])
                # sel[e, d] = one-hot of dst (edge-major layout)
                sel = selp.tile([P, nblk, P], BF16, tag="sel")
                nc.vector.tensor_tensor(
                    out=sel[:], in0=iota_t[:],
                    in1=dstl_sb[:, w * nblk:(w + 1) * nblk]
                    .to_broadcast([P, nblk, P]),
                    op=AL.is_equal,
                )
                # er expansion to edges: per block  erE = selt_b.T @ er_w
                erp = pser.tile([P, nblk * H], F32, tag="er")
                for b in range(nblk):
                    nc.tensor.matmul(
                        erp[:, b * H:(b + 1) * H],
                        lhsT=selt[:, b * P:(b + 1) * P],
                        rhs=herb[:, w, :],
                        start=True, stop=True,
                    )
                erb = mmp.tile([P, nblk, H], BF16, tag="erb")
                nc.vector.tensor_copy(erb[:], erp[:])
                # e = leaky_relu(el + er);  ex = exp(e)  (into zel's el slot)
                eall = mmp.tile([P, nblk, H], F32, tag="eall")
                nc.vector.tensor_tensor(
                    out=eall[:], in0=zel[:, :, HD:HD + H], in1=erb[:], op=AL.add
                )
                et = mmp.tile([P, nblk, H], F32, tag="et")
                nc.vector.tensor_scalar_mul(et[:], eall[:], NEG_SLOPE)
                nc.vector.tensor_tensor(out=eall[:], in0=eall[:], in1=et[:],
                                        op=AL.max)
                nc.scalar.activation(zel[:, :, HD:HD + H], eall[:], ACT_EXP)
                # msg = z * ex  (in place; two halves so scatters start early)
                h2 = nblk // 2
                for b0, b1 in ((0, h2), (h2, nblk)):
                    zb = zel[:, b0:b1, 0:HD].rearrange(
                        "p b (h d) -> p b h d", h=H)
                    nc.vector.tensor_tensor(
                        out=zb, in0=zb,
                        in1=zel[:, b0:b1, HD:HD + H]
                        .to_broadcast([P, b1 - b0, H, D]),
                        op=AL.mult,
                    )
                # scatter-add [msg | sum_ex] via one-hot matmuls
                outp = psout.tile([P, HD + H], F32, tag="out")
                for b in range(nblk):
                    nc.tensor.matmul(
                        outp[:],
                        lhsT=sel[:, b, :],
                        rhs=zel[:, b, 0:HD + H],
                        start=(b == 0),
                        stop=(b == nblk - 1),
                    )
                # normalize + residual + activation(s)
                rec = mmp.tile([P, H], F32, tag="rec")
                nc.vector.reciprocal(rec[:], outp[:, HD:HD + H])
                agg = mmp.tile([P, HD], F32, tag="agg")
                nc.vector.tensor_tensor(
                    out=agg[:].rearrange("p (h d) -> p h d", h=H),
                    in0=outp[:, 0:HD].rearrange("p (h d) -> p h d", h=H),
                    in1=rec[:].to_broadcast([P, H, D]),
                    op=AL.mult,
                )
                # elu(x) = Relu(x) + Exp(-Relu(-x)) - 1, mostly on the ACT
                # engine (tensor_scalar on DVE measures ~3.7x slower than
                # tensor_tensor, so the -1 uses a broadcast const add).
                mn = mmp.tile([P, HD], F32, tag="emn")
                ex = mmp.tile([P, HD], F32, tag="eex")
                if l == 0:
                    # h = elu(agg)
                    nc.scalar.activation(mn[:], agg[:], ACT_RELU, scale=-1.0)
                    nc.scalar.activation(ex[:], mn[:], ACT_EXP, scale=-1.0)
                    nc.scalar.activation(mn[:], agg[:], ACT_RELU)
                    nc.vector.tensor_tensor(out=ex[:], in0=ex[:], in1=mn[:],
                                            op=AL.add)
                    nc.vector.tensor_tensor(
                        out=h_all[:, w, :], in0=ex[:],
                        in1=neg1[:].to_broadcast([P, HD]), op=AL.add)
                else:
                    # h = elu(elu(agg + h))  (fused double elu):
                    #   n = Relu(-x); e = Exp(Exp(-n) - 1); h = Relu(x) + e - 1
                    nc.vector.tensor_tensor(out=agg[:], in0=agg[:],
                                            in1=h_all[:, w, :], op=AL.add)
                    nc.scalar.activation(mn[:], agg[:], ACT_RELU, scale=-1.0)
                    nc.scalar.activation(ex[:], mn[:], ACT_EXP, scale=-1.0)
                    nc.scalar.activation(ex[:], ex[:], ACT_EXP, bias=neg1[:])
                    nc.scalar.activation(mn[:], agg[:], ACT_RELU)
                    nc.vector.tensor_tensor(out=ex[:], in0=ex[:], in1=mn[:],
                                            op=AL.add)
                    nc.vector.tensor_tensor(
                        out=h_all[:, w, :], in0=ex[:],
                        in1=neg1[:].to_broadcast([P, HD]), op=AL.add)
                nc.vector.tensor_copy(hb_all[:, w, :], h_all[:, w, :])

            def pool_window(w):
                gp = pshg.tile([G, HD], F32, tag="hg")
                nc.tensor.matmul(
                    gp[:],
                    lhsT=gmk_sb[:, w * G:(w + 1) * G],
                    rhs=hb_all[:, w, :],
                    start=True, stop=True,
                )
                nc.vector.tensor_tensor(out=hg_acc[:], in0=hg_acc[:],
                                        in1=gp[:], op=AL.add)

            # ---- program ----
            wext = [build_wext(0, IN_DIM), build_wext(1, HD), build_wext(2, HD)]
            l0_table(wext[0])
            HW2 = NW // 2
            for l in range(3):
                for w in range(NW):
                    edge_window(l, w)
                    if l < 2:
                        slice_window(l + 1, wext[l + 1], w)
                    else:
                        pool_window(w)
                if l < 2:
                    nc.gpsimd.collective_compute(
                        "AllGather", AL.bypass,
                        replica_groups=[list(range(NCORES))],
                        ins=[zsl[l + 1][:, :]],
                        outs=[ztab[l + 1][:, :]],
                    )

            # ---- epilogue: AllReduce graph sums -> mean -> elu -> @Wc+bc ----
            nc.sync.dma_start(hgin[:, :], hg_acc[:])
            nc.gpsimd.collective_compute(
                "AllReduce", AL.add,
                replica_groups=[list(range(NCORES))],
                ins=[hgin[:, :]], outs=[hgout[:, :]],
            )
            hgr = smallp.tile([G, HD], F32, tag="hgr")
            nc.sync.dma_start(hgr[:], hgout[:, :])
            nc.vector.tensor_scalar_mul(hgr[:], hgr[:], 1.0 / NPG)
            mn = smallp.tile([G, HD], F32, tag="fmn")
            exx = smallp.tile([G, HD], F32, tag="fex")
            nc.vector.tensor_scalar_min(mn[:], hgr[:], 0.0)
            nc.scalar.activation(exx[:], mn[:], ACT_EXP)
            nc.vector.tensor_scalar_add(exx[:], exx[:], -1.0)
            nc.vector.tensor_scalar_max(mn[:], hgr[:], 0.0)
            nc.vector.tensor_tensor(out=hgr[:], in0=exx[:], in1=mn[:], op=AL.add)
            hgb = smallp.tile([G, HD], BF16, tag="hgb")
            nc.vector.tensor_copy(hgb[:], hgr[:])

            wc_sb, hgts = [], []
            for c2 in range(2):
                t = smallp.tile([P, C], BF16, tag="wc")
                nc.sync.dma_start(t[:], Wc[c2 * P:(c2 + 1) * P, :])
                wc_sb.append(t)
                tp = psmm.tile([P, G], BF16, tag="mm")
                nc.tensor.transpose(
                    tp[:], hgb[:, c2 * P:(c2 + 1) * P], ident[:G, :G]
                )
                hgt = smallp.tile([P, G], BF16, tag="hgt")
                nc.vector.tensor_copy(hgt[:], tp[:])
                hgts.append(hgt)
            lg = psmm.tile([G, C], F32, tag="mm")
            for c2 in range(2):
                nc.tensor.matmul(
                    lg[:], lhsT=hgts[c2][:], rhs=wc_sb[c2][:],
                    start=(c2 == 0), stop=(c2 == 1),
                )
            bc_sb = smallp.tile([G, C], F32, tag="bc")
            nc.sync.dma_start(bc_sb[:], bc[:, :])
            lg_sb = smallp.tile([G, C], F32, tag="lg")
            nc.vector.tensor_tensor(out=lg_sb[:], in0=lg[:], in1=bc_sb[:],
                                    op=AL.add)
            nc.sync.dma_start(logits[:, :], lg_sb[:])

    nc.compile()
    return nc


def _get_program(nblk):
    if nblk not in _CACHE:
        _CACHE[nblk] = _build_program(nblk)
    return _CACHE[nblk]


# ----------------------------------------------------------------------------
# Entry point
# ----------------------------------------------------------------------------
def kernel(x, src, dst, graph_ids, W0, al0, ar0, W1, al1, ar1, W2, al2, ar2, Wc, bc):
    global LAST_EXEC_NS, LAST_RESULTS
    x = np.ascontiguousarray(np.asarray(x, np.float32))
    src = np.asarray(src).astype(np.int64)
    dst = np.asarray(dst).astype(np.int64)
    graph_ids = np.asarray(graph_ids).astype(np.int64)

    nblk, xT, srcidx_d, dstloc_d, selth_d, gmask_d, xcT_d = _host_prep(
        x, src, dst, graph_ids)
    nc = _get_program(nblk)

    Wl = [np.asarray(W0, np.float32), np.asarray(W1, np.float32),
          np.asarray(W2, np.float32)]
    als = [al0, al1, al2]
    ars = [ar0, ar1, ar2]
    common = {
        "xT": xT,
        "Wc": np.asarray(Wc, np.float32).astype(BF),
        "bc_rep": np.tile(np.asarray(bc, np.float32)[None, :], (G, 1)),
    }
    for l in range(3):
        common[f"W{l}"] = Wl[l].astype(BF)
        common[f"WT{l}"] = np.ascontiguousarray(Wl[l].T).astype(BF)
        common[f"albd{l}"] = _blockdiag(np.asarray(als[l], np.float32)).astype(BF)
        common[f"arbd{l}"] = _blockdiag(np.asarray(ars[l], np.float32)).astype(BF)

    in_maps = []
    for c in range(NCORES):
        m = dict(common)
        m["xcT"] = xcT_d[c]
        m["srcidx"] = srcidx_d[c]
        m["dstloc"] = dstloc_d[c]
        m["selth"] = selth_d[c]
        m["gmask"] = gmask_d[c]
        in_maps.append(m)

    import os
    if os.environ.get("KERNEL_SIM"):
        from concourse.bass_interp import MultiCoreSim

        sim = MultiCoreSim(nc, num_cores=NCORES)
        for c, core_sim in sim.cores.items():
            for k, v in in_maps[c].items():
                core_sim.tensor(k)[:] = v
        sim.simulate()
        LAST_EXEC_NS = None
        return np.asarray(sim.cores[0].tensor("logits")).astype(np.float32)

    if TRACE:
        _install_ntff_hook_shim()
    res = run_bass_kernel_spmd(nc, in_maps, list(range(NCORES)), trace=TRACE)
    LAST_EXEC_NS = res.exec_time_ns
    LAST_RESULTS = res
    return res.results[0]["logits"].astype(np.float32)
